# revision 29
# baseline (speedup 1.0000x reference)
"""Trainium2 Bass kernel for CALayer with top-k channel masking.

Computation (per batch item):
  y = mean(x, spatial)                    # [C]
  h = relu(w1 @ y + b1)                   # [C/R]
  a = sigmoid(w2 @ h + b2)                # [C]
  idx = sort(top_k(a, 128).indices)       # ascending channel ids
  out = a[idx, None, None] * x[idx]       # [128, H, W]

Strategy: data-parallel over batch (32 items -> 8 cores x 4), f16 I/O.
x ships as f16 (halves HBM read traffic; verified: selection identical to the
f32 reference for these inputs, z-perturbation ~2e-6 vs min top-k boundary
gap 1.6e-5 -- bf16 flips selections, f16 does not). Outputs are written f16
and upcast on host (rel err ~3e-4 vs 2e-2 tolerance).

Measured machine facts this schedule is built around:
  - every reduction path (ACT Copy+accum, DVE accum, DVE tensor_reduce) runs
    at ~1 elem/cycle/partition (~0.9-1.1 ns/elem); no 16-bit fast mode. The
    32K-elem/core spatial-mean work is the dominant compute and must be split
    across ACT and DVE. GPSIMD elementwise is ~13 ns/elem (useless).
  - DVE tensor_scalar (mult) does hit a 2x mode with f16: ~0.3 ns/elem.
  - a dma_start costs ~0.7-1.2us of issue time on its engine's sequencer and
    the ring blocks when full, so all loads are issued from the otherwise
    idle SYNC engine, on ONE ring (rings are drained per-descriptor
    round-robin, so a second ring would starve the small const transfers).
  - engines execute in order; the per-engine instruction order below is
    hand-interleaved against data-arrival times so no engine stalls on a
    not-yet-landed batch while later work is available.

Pipeline: loads stream b0..b3 (the last batch at half-chunk granularity,
chunk1 first); per batch: means (ACT/DVE split), tiny-matmul MLP on PE, rank
top-k mask (transpose-broadcast + is_gt accum), slot indices via prefix-sum
matmul (strict-upper-triangular const, OOB fold), xs = x*sigmoid(z) on DVE,
and one indirect SBUF->DRAM scatter per (batch, chunk) with bounds_check
dropping unselected rows at descriptor level. Chunk slot ranges are disjoint
(ascending ids), each scatter targets its own output tensor; the host merges
with an exact add over zero-initialized buffers.
"""

from contextlib import ExitStack

import numpy as np

import concourse.bass as bass
import concourse.tile as tile
from concourse import bacc, mybir
from concourse.bass_utils import run_bass_kernel_spmd

N_CORES = 8
B_FULL, C, H, W = 32, 256, 64, 64
NB = B_FULL // N_CORES  # batch items per core
HW = H * W
HH = HW // 2  # half-chunk spatial extent
K = 128  # top-k
P = 128  # partitions
NCH = C // P  # channel chunks
R = 16  # reduction dim
OOB = 512.0  # out-of-bounds slot for unselected channels
F32 = mybir.dt.float32
F16 = mybir.dt.float16

# const blob A column layout (128 partitions, f32)
A_W1T = 0  # [P, NCH*R]  w1(+mean fold) transposed, chunk-major
A_B2 = A_W1T + NCH * R  # [P, NCH]
A_IDT = A_B2 + NCH  # [P, P] identity
A_END = A_IDT + P
# const blob C column layout (128 partitions, f16): small-integer matmul
# weights -- f16 weight loads/matmuls run ~4x faster on PE and the values
# (0/1/-OOB and the 0/1 mask rhs) are exactly representable
C_SUT = 0  # [P, P] strict-upper - OOB*I
C_ONE = C_SUT + P  # [P, P] ones
C_END = C_ONE + P
# const blob B column layout (16 partitions)
B_W2T = 0  # [R, C]
B_B1 = B_W2T + C  # [R, 1]
B_END = B_B1 + 1


def _body(ctx: ExitStack, tc: "tile.TileContext", x_d, outs_d, ca_d, cb_d, cc_d):
    nc = tc.nc
    AF = mybir.ActivationFunctionType
    ALU = mybir.AluOpType

    cpool = ctx.enter_context(tc.tile_pool(name="const", bufs=1))
    xp = ctx.enter_context(tc.tile_pool(name="x", bufs=NB))
    xsp = ctx.enter_context(tc.tile_pool(name="xs", bufs=2))
    sp = ctx.enter_context(tc.tile_pool(name="small", bufs=4))
    gp = ctx.enter_context(tc.tile_pool(name="g", bufs=2))
    pp = ctx.enter_context(tc.tile_pool(name="ps", bufs=2, space="PSUM"))
    zp = ctx.enter_context(tc.tile_pool(name="zrep", bufs=2, space="PSUM"))
    trA = ctx.enter_context(tc.tile_pool(name="trA", bufs=2))
    trD = ctx.enter_context(tc.tile_pool(name="trD", bufs=2))

    ca = cpool.tile([P, A_END], F32)
    cb = cpool.tile([R, B_END], F32)
    cc = cpool.tile([P, C_END], F16)

    # ---- DMA: one ring (sync engine), issue order == completion order ----
    nc.sync.dma_start(ca[:], ca_d.ap())
    nc.sync.dma_start(cb[:], cb_d.ap())
    nc.sync.dma_start(cc[:], cc_d.ap())
    xts = [xp.tile([P, NCH, HW], F16, tag="x", name=f"xt{b}") for b in range(NB)]
    LOAD_PIECES = [  # (batch, chunk, half-slice) in arrival order; b0 at
        # half granularity so ACT's means start the moment warmups finish
        (0, 0, slice(0, HH)), (0, 0, slice(HH, HW)),
        (0, 1, slice(0, HH)), (0, 1, slice(HH, HW)),
        (1, 0, slice(0, HW)), (1, 1, slice(0, HW)),
        (2, 0, slice(0, HW)), (2, 1, slice(0, HW)),
        (3, 1, slice(0, HW)), (3, 0, slice(0, HW)),
    ]
    for b, k, hs in LOAD_PIECES:
        nc.sync.dma_start(xts[b][:, k, hs], x_d.ap()[b][:, k * HW + hs.start : k * HW + hs.stop])

    # ---- warm ACT tables (Relu+Sigmoid) during the head ----
    wrm = cpool.tile([P, 2], F32)
    nc.scalar.activation(wrm[:, 0:1], ca[:, 0:1], AF.Relu)
    nc.scalar.activation(wrm[:, 1:2], ca[:, 0:1], AF.Sigmoid, accum_out=wrm[:, 0:1])

    y2s = [sp.tile([P, NCH, 2], F32, tag=f"y{b}", name=f"y2_{b}") for b in range(NB)]
    hsls = [[(0, 0), (0, 1), (1, 0), (1, 1)], [(0, 0), (1, 0)], [(0, 0), (1, 0)], [(0, 0), (0, 1), (1, 0), (1, 1)]]
    abq = {}

    def mean_act(b, k, hs, slot):
        t = trA.tile([P, HW], F16, tag="t")
        nc.scalar.activation(t[:, 0 : hs.stop - hs.start], xts[b][:, k, hs], AF.Copy, accum_out=y2s[b][:, k, slot : slot + 1])

    def mean_dve(b, k, hs, slot):
        t = trD.tile([P, HW], F16, tag="t")
        nc.vector.tensor_scalar(t[:, 0 : hs.stop - hs.start], xts[b][:, k, hs], 1.0, None, ALU.mult, ALU.add, accum_out=y2s[b][:, k, slot : slot + 1])

    def mlp(b):
        """PE matmuls + ACT relu/sigmoid: y2 -> ht -> z -> a (per batch)."""
        y2, hsl = y2s[b], hsls[b]
        ht_ps = pp.tile([R, 1], F32, tag="ht")
        for i, (k, h) in enumerate(hsl):
            nc.tensor.matmul(ht_ps[:], lhsT=ca[:, A_W1T + k * R : A_W1T + (k + 1) * R], rhs=y2[:, k, h : h + 1], start=(i == 0), stop=(i == len(hsl) - 1))
        ht_sb = sp.tile([R, 1], F32, tag="htsb")
        nc.scalar.activation(ht_sb[:], ht_ps[:], AF.Relu, bias=cb[:, B_B1 : B_B1 + 1])
        z_ps = pp.tile([P, NCH], F32, tag="z")
        for k in range(NCH):
            nc.tensor.matmul(z_ps[:, k : k + 1], lhsT=cb[:, B_W2T + k * P : B_W2T + (k + 1) * P], rhs=ht_sb[:], start=True, stop=True)
        a_sb = sp.tile([P, NCH], F32, tag="a")
        for k in range(NCH):
            nc.scalar.activation(a_sb[:, k : k + 1], z_ps[:, k : k + 1], AF.Sigmoid, bias=ca[:, A_B2 + k : A_B2 + k + 1])
        return z_ps, a_sb

    ms = {}

    def smallA(b, z_ps, a_sb):
        """DVE small ops + PE transposes: z -> rank -> mask."""
        zb_sb = sp.tile([P, NCH], F32, tag="zb")
        nc.vector.tensor_tensor(out=zb_sb[:], in0=z_ps[:], in1=ca[:, A_B2 : A_B2 + NCH], op=ALU.add)
        zrep_ps = zp.tile([P, C], F32, tag="zrep")
        for k in range(NCH):
            nc.tensor.transpose(zrep_ps[:, k * P : (k + 1) * P], in_=zb_sb[:, k : k + 1].to_broadcast([P, P]), identity=ca[:, A_IDT : A_IDT + P])
        rank = sp.tile([P, NCH], F32, tag="rank")
        for k in range(NCH):
            g = gp.tile([P, C], F32, tag="g")
            nc.vector.tensor_scalar(g[:], zrep_ps[:], zb_sb[:, k : k + 1], None, ALU.is_gt, ALU.add, accum_out=rank[:, k : k + 1])
        m = sp.tile([P, NCH], F16, tag="m")
        nc.vector.tensor_scalar(m[:], rank[:], float(K) - 0.5, None, ALU.is_lt)
        ms[b] = (m, a_sb)
        xs = xsp.tile([P, NCH, HW], F16, tag="xs")
        abq[b] = [a_sb, None, xs]

    def smallB(b):
        """PE prefix-sum + DVE slot fuse: mask -> scatter slot indices."""
        m, a_sb = ms.pop(b)
        p_ps = pp.tile([P, NCH + 1], F32, tag="p")
        nc.tensor.matmul(p_ps[:, 0:NCH], lhsT=cc[:, C_SUT : C_SUT + P], rhs=m[:, 0:NCH], start=True, stop=True)
        nc.tensor.matmul(p_ps[:, NCH : NCH + 1], lhsT=cc[:, C_ONE : C_ONE + P], rhs=m[:, 0:1], start=True, stop=True)
        qi = sp.tile([P, NCH], mybir.dt.int32, tag="qi")
        nc.vector.tensor_scalar(qi[:, 0:1], p_ps[:, 0:1], OOB, None, ALU.add)
        nc.vector.tensor_scalar(qi[:, 1:2], p_ps[:, 1:2], p_ps[:, NCH : NCH + 1], OOB, ALU.add, ALU.add)
        abq[b][1] = qi

    def xs_dve(b, k):
        a_sb, _, xs = abq[b]
        nc.vector.tensor_scalar(xs[:, k, :], xts[b][:, k, :], a_sb[:, k : k + 1], None, ALU.mult)

    def xs_act(b, k):
        a_sb, _, xs = abq[b]
        nc.scalar.activation(xs[:, k, :], xts[b][:, k, :], AF.Copy, scale=a_sb[:, k : k + 1])

    def scat(b, k):
        _, qi, xs = abq[b]
        nc.gpsimd.indirect_dma_start(
            out=outs_d[b][k].ap(),
            out_offset=bass.IndirectOffsetOnAxis(ap=qi[:, k : k + 1], axis=0),
            in_=xs[:, k, :],
            in_offset=None,
            bounds_check=K - 1,
            oob_is_err=False,
        )

    # ---- hand-interleaved schedule (per-engine order == emission order) ----
    HSn = slice(0, HW)
    h0, h1 = slice(0, HH), slice(HH, HW)
    # PE is in-order and its per-batch block (MLP matmuls, transposes, prefix
    # matmuls with weight reloads) was the chain serializer: each batch's MLP
    # sat behind the previous batch's prefix. The order below pushes every
    # prefix (smallB) behind the NEXT batch's MLP, and runs b3 (whose data
    # lands before b2's last mean finishes) to completion before b2.
    mean_act(0, 0, h0, 0)    # A: b0 quarters -- entirely on ACT so its MLP
    mean_act(0, 0, h1, 1)    #    never waits on a cross-engine mean, and the
    mean_act(0, 1, h0, 0)    #    first quarter lands before the warmups end
    mean_act(0, 1, h1, 1)
    z0, a0 = mlp(0)          # PE/ACT
    mean_dve(1, 1, HSn, 0)   # D: b1c1 (D's first work)
    smallA(0, z0, a0)        # D/PE: zb, transpose, rank, mask
    mean_act(1, 0, HSn, 0)   # A: b1c0
    z1, a1 = mlp(1)          # PE: ht1/z1 ahead of prefix0
    smallB(0)                # PE prefix0 + DVE qi0
    xs_dve(0, 0); xs_dve(0, 1); scat(0, 0); scat(0, 1)
    smallA(1, z1, a1)
    mean_act(2, 0, HSn, 0)   # A: b2c0
    smallB(1)
    xs_dve(1, 0); xs_dve(1, 1); scat(1, 0); scat(1, 1)
    mean_dve(3, 1, h0, 0)    # D: b3c1h0 (arrives before b3c0)
    mean_act(3, 1, h1, 1)    # A: b3c1h1
    mean_act(3, 0, h0, 0)    # A: b3c0h0
    mean_dve(3, 0, h1, 1)    # D: b3c0h1
    z3, a3 = mlp(3)          # b3's stats complete before b2's last mean
    smallA(3, z3, a3)
    smallB(3)
    mean_act(2, 1, HSn, 0)   # A: b2c1 (latest-needed mean -> last on ACT)
    xs_dve(3, 0); scat(3, 0); xs_dve(3, 1); scat(3, 1)
    z2, a2 = mlp(2)
    smallA(2, z2, a2)
    xs_dve(2, 1)             # needs only a2, runs while prefix2 is on PE
    smallB(2)
    xs_act(2, 0)             # A helps with the final batch's scaling
    scat(2, 0); scat(2, 1)


def build_nc():
    nc = bacc.Bacc("TRN2", target_bir_lowering=False, debug=False, num_devices=N_CORES)
    x_d = nc.dram_tensor("x", [NB, P, NCH * HW], F16, kind="ExternalInput")
    ca_d = nc.dram_tensor("ca", [P, A_END], F32, kind="ExternalInput")
    cb_d = nc.dram_tensor("cb", [R, B_END], F32, kind="ExternalInput")
    cc_d = nc.dram_tensor("cc", [P, C_END], F16, kind="ExternalInput")
    outs_d = [[nc.dram_tensor(f"out{b}c{k}", [K, HW], F16, kind="ExternalOutput") for k in range(NCH)] for b in range(NB)]
    with tile.TileContext(nc) as tc:
        with ExitStack() as ctx:
            _body(ctx, tc, x_d, outs_d, ca_d, cb_d, cc_d)
    nc.compile()
    return nc


def make_in_maps(x, w1, b1, w2, b2):
    """Per-core input dicts. x: [32, 256, 64, 64] f32."""
    w1t = np.ascontiguousarray(w1.T).astype(np.float32) / float(HW)  # [C, R], mean folded in
    ca = np.zeros((P, A_END), np.float32)
    ca[:, A_W1T : A_W1T + NCH * R] = w1t.reshape(NCH, P, R).transpose(1, 0, 2).reshape(P, NCH * R)
    ca[:, A_B2 : A_B2 + NCH] = b2.astype(np.float32).reshape(NCH, P).T
    ca[:, A_IDT : A_IDT + P] = np.eye(P, dtype=np.float32)
    cc = np.zeros((P, C_END), np.float16)
    cc[:, C_SUT : C_SUT + P] = np.triu(np.ones((P, P), np.float16), k=1) - np.float16(OOB) * np.eye(P, dtype=np.float16)
    cc[:, C_ONE : C_ONE + P] = 1.0
    cb = np.zeros((R, B_END), np.float32)
    cb[:, B_W2T : B_W2T + C] = np.ascontiguousarray(w2.T).astype(np.float32)
    cb[:, B_B1] = b1.astype(np.float32)
    # partition-contiguous layout: [B, P, NCH*HW], partition p holds channels
    # (p, p+128) back to back -- one 16KB descriptor per partition per batch
    xr = x.astype(np.float16).reshape(B_FULL, NCH, P, HW).transpose(0, 2, 1, 3).reshape(B_FULL, P, NCH * HW)
    in_maps = []
    for i in range(N_CORES):
        in_maps.append(
            {
                "x": np.ascontiguousarray(xr[i * NB : (i + 1) * NB]),
                "ca": ca,
                "cb": cb,
                "cc": cc,
            }
        )
    return in_maps


def _install_ntff_hook():
    """Bridge the missing antenv.axon_hooks module so run_bass_kernel_spmd
    trace=True can capture NTFF profiles via the axon PJRT .so."""
    import sys
    import types

    if "antenv.axon_hooks" in sys.modules:
        return
    try:
        if "/root/.axon_site" not in sys.path:
            sys.path.insert(0, "/root/.axon_site")
        from trn_agent_boot.trn_boot import _ntff_profile_via_ctypes

        hook = _ntff_profile_via_ctypes("/opt/axon/libaxon_pjrt.so")
        mod = types.ModuleType("antenv.axon_hooks")
        mod.get_axon_ntff_profile_hook = lambda: hook
        mod.set_axon_ntff_profile_hook = lambda h: None
        sys.modules["antenv.axon_hooks"] = mod
    except Exception as e:  # degrade to no tracing
        print("ntff hook install failed:", e)


_NC_CACHE = {}


def get_nc():
    if "nc" not in _NC_CACHE:
        _NC_CACHE["nc"] = build_nc()
    return _NC_CACHE["nc"]


def kernel(x, w1, b1, w2, b2, topk, _trace=False, **_ignored):
    assert int(topk) == K, f"kernel hardcodes topk={K}, got {topk}"
    assert x.shape == (B_FULL, C, H, W)
    nc = get_nc()
    if _trace:
        _install_ntff_hook()
    in_maps = make_in_maps(np.asarray(x), np.asarray(w1), np.asarray(b1), np.asarray(w2), np.asarray(b2))
    res = run_bass_kernel_spmd(nc, in_maps, core_ids=list(range(N_CORES)), trace=_trace)
    # chunk scatters write disjoint slot ranges of each batch's output into
    # separate zero-initialized tensors; merging them is an exact add
    outs = [
        np.stack([res.results[i][f"out{b}c0"].astype(np.float32) + res.results[i][f"out{b}c1"].astype(np.float32) for b in range(NB)]).reshape(NB, K, H, W)
        for i in range(N_CORES)
    ]
    full = np.concatenate(outs, axis=0).astype(np.float32)
    if _trace:
        return full, res
    return full
